# revision 30
# baseline (speedup 1.0000x reference)
"""Trainium2 Bass kernel for nn_FPSWE_40303973105696.

Computation (see problem reference): project X onto P directions, sort along
N, linearly interpolate N->M quantiles, subtract from ref, contract with
weight.

For these shapes (N=2048, M=1024) the quantile-interpolation gather is exactly
ind[m] = 2m, so the folded contraction weights are a simple even/odd
interleave of weight scaled by the interpolation fraction t[m]:

    out[b, p] = rw[p] - sum_n Xs[b, n, p] * W2[p, n]
    W2[p, 2m]   = weight[p, m] * (1 - t[m])
    W2[p, 2m+1] = weight[p, m] * t[m]
    rw[p]       = sum_m ref[m] * weight[p, m]

so W2 is built on-device from weight itself (no precomputed [P, N] upload).

Device kernel per core (data-parallel over B, core c handles batch c):
    1. transpose X[b] and theta_w via PE identity matmuls (inputs ship in
       natural layout as bf16; no host-side transposes)
    2. proj[p, n] = theta_w[p, :] @ X[b].T        (PE, bf16 -> fp32 PSUM)
    3. sort proj rows along n (free axis)         (bitonic bf16, 66 stages)
    4. W2 rows from weight + t                    (GPSIMD, overlapped)
    5. acc[p] = sum_n Xs[p, n] * W2[p, n]         (GPSIMD mult + ACT accum)
    6. out[p] = rw[p] - acc[p]

The sort keeps row-groups in wide SBUF buffers [128, G*2048] so each
compare-exchange stage is a single min + single max tensor_tensor op over the
whole group (blocks tile uniformly because 2048 % block == 0).
"""

import numpy as np

from concourse import bass, bacc, mybir
from concourse.tile import TileContext

B, N, D, P, M = 8, 2048, 128, 1024, 1024
NT = P // 128          # 8 projection row-chunks of 128 partitions each
MM_CHUNK = 512         # matmul free-dim chunk (one PSUM bank)
N_CORES = 8

# row-groups: list of (num_row_chunks, engine_name). Sum must be NT.
# NOTE: only DVE can sort — walrus rejects min/max TensorTensor on Pool.
GROUPS = [(NT, "vector")]

# debug knob: limit number of sort stages emitted (None = all)
STAGE_LIMIT = None
# benchmark knob: emit the whole kernel body this many times (timing only)
REPEAT = 1

FP = mybir.dt.float32
SD = mybir.dt.bfloat16  # sort dtype (bf16 -> DVE 2x perf mode on most stages)

# input names that are per-core (sharded); everything else is replicated
_SHARDED = {"xnat"}


def _sort_stages(n):
    """Normalized bitonic network with odd-block pre-reversal. 66 stages.

    The DVE bf16 2x perf mode needs every operand AP at <=2 free dims with
    forward unit steps (measured: reversed reads and 3-free-dim APs drop to
    1x). The stride-1 substage closing each merge level is 1x regardless, so
    it is parity-split to write odd blocks mirrored ("st1r") — the next
    level's mirror then reads both halves FORWARD ("mirror_r"): its min op
    earns 2x and only its max op keeps a (1x) reversed write.
    """
    stages = [("m2r", 2)]
    size = 4
    while size <= n:
        stages.append(("mirror_r", size))
        st = size // 4
        while st >= 2:
            stages.append(("std", st))
            st //= 2
        if size < n:
            stages.append(("st1r", size))
        else:
            stages.append(("std", 1))
        size *= 2
    return stages


def _emit_sort_stage(eng, kind, val, cur, oth):
    """One compare-exchange stage: read cur AP, write oth AP (same shape)."""
    mn, mx = mybir.AluOpType.min, mybir.AluOpType.max
    if kind == "m2r":
        # pair exchange; odd pairs stored (max, min) for mirror_r(4)
        v = cur.rearrange("p (g d t) -> p g d t", d=2, t=2)
        o = oth.rearrange("p (g d t) -> p g d t", d=2, t=2)
        eng.tensor_tensor(o[:, :, 0, 0], v[:, :, 0, 0], v[:, :, 0, 1], op=mn)
        eng.tensor_tensor(o[:, :, 0, 1], v[:, :, 0, 0], v[:, :, 0, 1], op=mx)
        eng.tensor_tensor(o[:, :, 1, 0], v[:, :, 1, 0], v[:, :, 1, 1], op=mx)
        eng.tensor_tensor(o[:, :, 1, 1], v[:, :, 1, 0], v[:, :, 1, 1], op=mn)
    elif kind == "mirror_r":
        s = val
        half = s // 2
        v = cur.rearrange("p (n s) -> p n s", s=s)
        o = oth.rearrange("p (n s) -> p n s", s=s)
        lo, upr = v[:, :, :half], v[:, :, half:]     # upr is pre-reversed
        eng.tensor_tensor(o[:, :, :half], lo, upr, op=mn)
        eng.tensor_tensor(o[:, :, half:][:, :, ::-1], lo, upr, op=mx)
    elif kind == "st1r":
        # stride-1 exchange; odd val-blocks written mirrored
        s = val
        v = cur.rearrange("p (g d b t) -> p g d b t", d=2, b=s // 2, t=2)
        o = oth.rearrange("p (g d b t) -> p g d b t", d=2, b=s // 2, t=2)
        eng.tensor_tensor(o[:, :, 0, :, 0], v[:, :, 0, :, 0],
                          v[:, :, 0, :, 1], op=mn)
        eng.tensor_tensor(o[:, :, 0, :, 1], v[:, :, 0, :, 0],
                          v[:, :, 0, :, 1], op=mx)
        eng.tensor_tensor(o[:, :, 1, ::-1, 1], v[:, :, 1, :, 0],
                          v[:, :, 1, :, 1], op=mn)
        eng.tensor_tensor(o[:, :, 1, ::-1, 0], v[:, :, 1, :, 0],
                          v[:, :, 1, :, 1], op=mx)
    else:
        st = val
        v = cur.rearrange("p (n t s) -> p n t s", t=2, s=st)
        o = oth.rearrange("p (n t s) -> p n t s", t=2, s=st)
        eng.tensor_tensor(o[:, :, 0, :], v[:, :, 0, :], v[:, :, 1, :],
                          op=mn)
        eng.tensor_tensor(o[:, :, 1, :], v[:, :, 0, :], v[:, :, 1, :],
                          op=mx)


def _build_kernel():
    assert sum(g for g, _ in GROUPS) == NT
    nc = bacc.Bacc()

    xnat = nc.declare_dram_parameter("xnat", [N, D], SD, isOutput=False)   # X[b]
    thn = nc.declare_dram_parameter("thn", [P, D], SD, isOutput=False)     # theta_w
    wq = nc.declare_dram_parameter("wq", [P, M], SD, isOutput=False)       # weight
    tv = nc.declare_dram_parameter("tv", [128, M], FP, isOutput=False)     # t[m]
    eye = nc.declare_dram_parameter("eye", [128, 128], SD, isOutput=False)
    rw = nc.declare_dram_parameter("rw", [128, NT], FP, isOutput=False)    # rw[p]
    out = nc.declare_dram_parameter("out", [128, NT], FP, isOutput=True)

    stages = _sort_stages(N)
    if STAGE_LIMIT is not None:
        stages = stages[:STAGE_LIMIT]

    with TileContext(nc) as tc:
        with (
            tc.tile_pool(name="const", bufs=1) as const_pool,
            tc.tile_pool(name="xt", bufs=1) as xt_pool,
            tc.tile_pool(name="sa", bufs=1) as a_pool,
            tc.tile_pool(name="sb", bufs=1) as b_pool,
            tc.tile_pool(name="wq", bufs=2) as w_pool,
            tc.tile_pool(name="w2", bufs=1) as w2_pool,
            tc.tile_pool(name="prod", bufs=2) as prod_pool,
            tc.tile_pool(name="pst", bufs=2, space="PSUM") as pst_pool,
            tc.tile_pool(name="ps", bufs=1, space="PSUM") as psum_pool,
        ):
            eye_raw = const_pool.tile([128, 128], SD, tag="eyer")
            eye_sb = const_pool.tile([128, 128], SD, tag="eye")
            tv_sb = const_pool.tile([128, M], FP, tag="tv")
            tvb_sb = const_pool.tile([128, M], FP, tag="tvb")
            rw_sb = const_pool.tile([128, NT], FP, tag="rw")
            acc_sb = const_pool.tile([128, NT], FP, tag="acc")
            out_sb = const_pool.tile([128, NT], FP, tag="outsb")
            out_tmp = const_pool.tile([128, NT], FP, tag="outtmp")
            xn_sb = xt_pool.tile([128, N], SD, tag="xn")        # X[b] natural
            xt_sb = xt_pool.tile([D, N], SD, tag="xt")          # X[b].T
            thn_sb = xt_pool.tile([128, P], SD, tag="thn")      # theta_w natural
            tht_sb = xt_pool.tile([D, P], SD, tag="tht")        # theta_w.T

            nc.sync.dma_start(out=eye_raw[:], in_=eye[:])
            nc.sync.dma_start(out=tv_sb[:], in_=tv[:])
            nc.sync.dma_start(out=rw_sb[:], in_=rw[:])
            # X[b] as [128, 16*128]: xn_sb[p, k*128+d] = X[k*128+p, d]
            nc.sync.dma_start(
                out=xn_sb.rearrange("p (k d) -> p k d", d=D),
                in_=xnat.rearrange("(k p) d -> p k d", p=128))
            # theta_w as [128, 8*128]: thn_sb[p, r*128+d] = theta_w[r*128+p, d]
            nc.sync.dma_start(
                out=thn_sb.rearrange("p (r d) -> p r d", d=D),
                in_=thn.rearrange("(r p) d -> p r d", p=128))
            # Bounce eye through ACT so Matmult (transpose) instructions never
            # carry two DMA-queue semaphore waits (walrus codegen limit).
            nc.scalar.copy(out=eye_sb[:], in_=eye_raw[:])
            # bounce tv through ACT so gpsimd W2-build ops carry at most one
            # DMA-queue semaphore wait
            nc.scalar.copy(out=tvb_sb[:], in_=tv_sb[:])

            # on-device transposes: X[b].T [D, N] and theta_w.T [D, P]
            for k in range(N // 128):
                ps = pst_pool.tile([128, 128], SD, tag="pst", name="pst")
                nc.tensor.transpose(
                    ps[:], xn_sb[:, k * 128:(k + 1) * 128], eye_sb[:])
                nc.scalar.copy(out=xt_sb[:, k * 128:(k + 1) * 128], in_=ps[:])
            for r in range(NT):
                ps = pst_pool.tile([128, 128], SD, tag="pst", name="pst")
                nc.tensor.transpose(
                    ps[:], thn_sb[:, r * 128:(r + 1) * 128], eye_sb[:])
                nc.scalar.copy(out=tht_sb[:, r * 128:(r + 1) * 128], in_=ps[:])

            # W2 rows prebuilt on GPSIMD (overlaps the sort): even slots
            # weight*(1-t), odd slots weight*t
            w2_full = w2_pool.tile([128, NT * N], FP, tag="w2full")
            for t in range(NT):
                w_sb = w_pool.tile([128, M], SD, tag="wq", name="wq")
                nc.sync.dma_start(out=w_sb[:],
                                  in_=wq[t * 128:(t + 1) * 128, :])
                w2v = w2_full[:, t * N:(t + 1) * N].rearrange(
                    "p (m q) -> p m q", q=2)
                nc.gpsimd.tensor_mul(w2v[:, :, 1], w_sb[:], tvb_sb[:])
                nc.gpsimd.tensor_sub(w2v[:, :, 0], w_sb[:], w2v[:, :, 1])

            a_tiles, b_tiles = {}, {}
            for gi, (gsz, eng_name) in enumerate(GROUPS):
                sd = SD if eng_name == "vector" else FP
                a_tiles[gi] = a_pool.tile([128, gsz * N], sd,
                                          tag=f"a{gi}", name=f"a{gi}")
                b_tiles[gi] = b_pool.tile([128, gsz * N], sd,
                                          tag=f"b{gi}", name=f"b{gi}")

            def emit_body(rep_i):
                # ---- phase A: projection matmuls for every chunk (PE+ACT) --
                t0 = 0
                for gi, (gsz, _) in enumerate(GROUPS):
                    a_t = a_tiles[gi]
                    for r in range(gsz):
                        t = t0 + r
                        ps = psum_pool.tile([128, N], FP, tag="ps", name="ps")
                        for ch in range(N // MM_CHUNK):
                            nc.tensor.matmul(
                                ps[:, ch * MM_CHUNK:(ch + 1) * MM_CHUNK],
                                lhsT=tht_sb[:, t * 128:(t + 1) * 128],
                                rhs=xt_sb[:, ch * MM_CHUNK:(ch + 1) * MM_CHUNK],
                                start=True, stop=True,
                            )
                        nc.scalar.copy(out=a_t[:, r * N:(r + 1) * N], in_=ps[:])
                    t0 += gsz

                # ---- phase B: per-group sort + weighted reduction ----------
                # First and last stages are emitted per row-chunk so the sort
                # can start as soon as each chunk's matmul lands, and each
                # chunk's weighted reduction can start right after its own
                # final compare-exchange.
                t0 = 0
                for gi, (gsz, eng_name) in enumerate(GROUPS):
                    a_t, b_t = a_tiles[gi], b_tiles[gi]
                    eng = getattr(nc, eng_name)
                    cur, oth = a_t[:], b_t[:]
                    for si, stg in enumerate(stages):
                        first, last = si == 0, si == len(stages) - 1
                        if first or last:
                            for r in range(gsz):
                                sl = slice(r * N, (r + 1) * N)
                                _emit_sort_stage(eng, stg[0], stg[1],
                                                 cur[:, sl], oth[:, sl])
                                if not last:
                                    continue
                                t = t0 + r
                                w2sl = w2_full[:, t * N:(t + 1) * N]
                                # HW-proven finals: gpsimd mult + ACT accum
                                # (tensor_tensor_reduce compiles but crashes
                                # the core at runtime)
                                prod = prod_pool.tile([128, N], FP,
                                                      tag="prod",
                                                      name="prod")
                                nc.gpsimd.tensor_mul(prod[:], oth[:, sl],
                                                     w2sl)
                                nc.scalar.activation(
                                    cur[:, sl], prod[:],
                                    mybir.ActivationFunctionType.Copy,
                                    accum_out=acc_sb[:, t:t + 1])
                        else:
                            _emit_sort_stage(eng, stg[0], stg[1], cur, oth)
                        cur, oth = oth, cur
                    t0 += gsz

                # accumulate across repeat bodies so none is dead code;
                # the final output is REPEAT * (rw - acc), divided on host
                if rep_i == 0:
                    nc.vector.tensor_sub(out_sb[:], rw_sb[:], acc_sb[:])
                else:
                    nc.vector.tensor_sub(out_tmp[:], rw_sb[:], acc_sb[:])
                    nc.vector.tensor_add(out_sb[:], out_sb[:], out_tmp[:])

            for _rep in range(REPEAT):
                emit_body(_rep)
            nc.sync.dma_start(out=out[:], in_=out_sb[:])

    return nc


_NC_CACHE = None


def _get_nc():
    global _NC_CACHE
    if _NC_CACHE is None:
        nc = _build_kernel()
        nc.finalize()   # Bacc: runs wait-splitting + register allocation
        _NC_CACHE = nc
    return _NC_CACHE


def _host_precompute(X, theta_w, ref, weight):
    """Global (all-core) input arrays, keyed by dram parameter name."""
    X = np.ascontiguousarray(np.asarray(X, dtype=np.float32))
    theta_w = np.ascontiguousarray(np.asarray(theta_w, dtype=np.float32))
    ref = np.asarray(ref, dtype=np.float32)
    weight = np.ascontiguousarray(np.asarray(weight, dtype=np.float32))

    x1d = np.linspace(0.0, 1.0, N + 2, dtype=np.float32)[1:-1]
    xnew = np.linspace(0.0, 1.0, M + 2, dtype=np.float32)[1:-1]
    ind = 2 * np.arange(M)      # == clip(searchsorted(x1d, xnew) - 1, 0, N-2)
    eps = np.float32(np.finfo(np.float32).eps)
    dx = x1d[1:] - x1d[:-1]
    t = ((xnew - x1d[ind]) / (eps + dx[ind])).astype(np.float32)

    rw = (weight.astype(np.float64) @ ref.astype(np.float64)).astype(np.float32)

    import ml_dtypes
    bf = ml_dtypes.bfloat16
    return {
        "xnat": X.reshape(B * N, D).astype(bf),           # sharded over cores
        "thn": theta_w.astype(bf),
        "wq": weight.astype(bf),
        "tv": np.ascontiguousarray(np.broadcast_to(t, (128, M))),
        "eye": np.eye(128, dtype=np.float32).astype(bf),
        "rw": np.ascontiguousarray(rw.reshape(NT, 128).T),
    }


# ---------------------------------------------------------------------------
# Cached-jit executor: trace/compile once per process, then each call is
# host->device of ~12 MB + one dispatch. Replicated inputs use P(None) so a
# single copy is shipped rather than 8.
# ---------------------------------------------------------------------------

_EXEC_CACHE = {}


def _get_exec():
    key = REPEAT
    if key in _EXEC_CACHE:
        return _EXEC_CACHE[key]
    import jax
    from jax.sharding import Mesh, PartitionSpec
    from jax.experimental.shard_map import shard_map
    from concourse import mybir as _mybir
    from concourse.bass2jax import (
        _bass_exec_p, install_neuronx_cc_hook, partition_id_tensor,
    )

    install_neuronx_cc_hook()
    nc = _get_nc()

    partition_name = (nc.partition_id_tensor.name
                      if nc.partition_id_tensor else None)
    in_names, out_names, out_avals, zero_outs = [], [], [], []
    for alloc in nc.m.functions[0].allocations:
        if not isinstance(alloc, _mybir.MemoryLocationSet):
            continue
        name = alloc.memorylocations[0].name
        if alloc.kind == "ExternalInput":
            if name == partition_name:
                continue
            in_names.append(name)
        elif alloc.kind == "ExternalOutput":
            out_names.append(name)
            shape = tuple(alloc.tensor_shape)
            dtype = _mybir.dt.np(alloc.dtype)
            out_avals.append(jax.core.ShapedArray(shape, dtype))
            zero_outs.append(np.zeros(shape, dtype))
    n_params = len(in_names)
    all_names = in_names + out_names
    if partition_name is not None:
        all_names = all_names + [partition_name]

    def _body(*args):
        operands = list(args)
        if partition_name is not None:
            operands.append(partition_id_tensor())
        outs = _bass_exec_p.bind(
            *operands,
            out_avals=tuple(out_avals),
            in_names=tuple(all_names),
            out_names=tuple(out_names),
            lowering_input_output_aliases=(),
            sim_require_finite=True,
            sim_require_nnan=True,
            nc=nc,
        )
        return tuple(outs)

    devices = jax.devices()[:N_CORES]
    mesh = Mesh(np.asarray(devices), ("core",))
    in_specs = tuple(
        PartitionSpec("core") if nm in _SHARDED else PartitionSpec()
        for nm in in_names
    ) + (PartitionSpec("core"),) * len(zero_outs)
    out_specs = (PartitionSpec("core"),) * len(out_names)
    fn = jax.jit(
        shard_map(_body, mesh=mesh,
                  in_specs=in_specs, out_specs=out_specs,
                  check_rep=False),
        keep_unused=True,
    )
    shardings = [jax.sharding.NamedSharding(mesh, sp) for sp in in_specs]
    res = (fn, in_names, out_names, zero_outs, shardings)
    _EXEC_CACHE[key] = res
    return res


def _run_jit(glob_inputs):
    import jax
    fn, in_names, out_names, zero_outs, shardings = _get_exec()
    args = [glob_inputs[nm] for nm in in_names]
    args += [np.zeros((N_CORES * z.shape[0], *z.shape[1:]), z.dtype)
             for z in zero_outs]
    # one batched H2D (per-array puts each pay an axon roundtrip)
    try:
        dargs = jax.device_put(args, shardings)
    except Exception:
        dargs = args
    outs = fn(*dargs)
    jax.block_until_ready(outs)
    return {nm: np.asarray(o) for nm, o in zip(out_names, outs)}


def _run_fallback(glob_inputs):
    """Per-core run via run_bass_kernel_spmd (native-NRT capable path)."""
    from concourse.bass_utils import run_bass_kernel_spmd
    nc = _get_nc()
    in_maps = []
    for c in range(N_CORES):
        m = {}
        for nm, arr in glob_inputs.items():
            if nm in _SHARDED:
                sh = arr.shape[0] // N_CORES
                m[nm] = arr[c * sh:(c + 1) * sh]
            else:
                m[nm] = arr
        in_maps.append(m)
    res = run_bass_kernel_spmd(nc, in_maps, list(range(N_CORES)))
    outs = res.results if hasattr(res, "results") else res
    return {"out": np.concatenate([o["out"] for o in outs], axis=0)}


def _assemble(out_concat):
    out_full = np.empty((B, P), dtype=np.float32)
    for c in range(N_CORES):
        o = out_concat[c * 128:(c + 1) * 128]
        out_full[c] = np.ascontiguousarray(o.T).reshape(P)
    return out_full / REPEAT


def kernel(X, theta_w, ref, weight):
    import time as _time

    glob_inputs = _host_precompute(X, theta_w, ref, weight)
    last_err = None
    for attempt in range(3):
        try:
            outs = _run_jit(glob_inputs)
            return _assemble(outs["out"])
        except Exception as e:  # transient transport errors (mesh desync)
            last_err = e
            _time.sleep(3)
    # final fallback: sanctioned spmd runner (works native or axon)
    try:
        outs = _run_fallback(glob_inputs)
        return _assemble(outs["out"])
    except Exception:
        raise last_err


# ---------------------------------------------------------------------------
# Benchmark path: cached jit + device-resident inputs, excludes host transfer.
# ---------------------------------------------------------------------------

def make_bench(X, theta_w, ref, weight):
    import jax

    fn, in_names, out_names, zero_outs, _shardings = _get_exec()
    glob_inputs = _host_precompute(X, theta_w, ref, weight)
    args = [glob_inputs[nm] for nm in in_names]
    args += [np.zeros((N_CORES * z.shape[0], *z.shape[1:]), z.dtype)
             for z in zero_outs]
    dev_in = [jax.device_put(a) for a in args]

    def run():
        outs = fn(*dev_in)
        jax.block_until_ready(outs)
        return outs

    def collect(outs):
        return _assemble(np.asarray(outs[0]))

    return run, collect


# revision 31
# speedup vs baseline: 1.0851x; 1.0851x over previous
"""Trainium2 Bass kernel for nn_FPSWE_40303973105696.

Computation (see problem reference): project X onto P directions, sort along
N, linearly interpolate N->M quantiles, subtract from ref, contract with
weight.

For these shapes (N=2048, M=1024) the quantile-interpolation gather is exactly
ind[m] = 2m, so the folded contraction weights are a simple even/odd
interleave of weight scaled by the interpolation fraction t[m]:

    out[b, p] = rw[p] - sum_n Xs[b, n, p] * W2[p, n]
    W2[p, 2m]   = weight[p, m] * (1 - t[m])
    W2[p, 2m+1] = weight[p, m] * t[m]
    rw[p]       = sum_m ref[m] * weight[p, m]

so W2 is built on-device from weight itself (no precomputed [P, N] upload).

Device kernel per core (data-parallel over B, core c handles batch c):
    1. transpose X[b] and theta_w via PE identity matmuls (inputs ship in
       natural layout as bf16; no host-side transposes)
    2. proj[p, n] = theta_w[p, :] @ X[b].T        (PE, bf16 -> fp32 PSUM)
    3. sort proj rows along n (free axis)         (bitonic bf16, 66 stages)
    4. W2 rows from weight + t                    (GPSIMD, overlapped)
    5. acc[p] = sum_n Xs[p, n] * W2[p, n]         (GPSIMD mult + ACT accum)
    6. out[p] = rw[p] - acc[p]

The sort keeps row-groups in wide SBUF buffers [128, G*2048] so each
compare-exchange stage is a single min + single max tensor_tensor op over the
whole group (blocks tile uniformly because 2048 % block == 0).
"""

import numpy as np

from concourse import bass, bacc, mybir
from concourse.tile import TileContext

B, N, D, P, M = 8, 2048, 128, 1024, 1024
NT = P // 128          # 8 projection row-chunks of 128 partitions each
MM_CHUNK = 512         # matmul free-dim chunk (one PSUM bank)
N_CORES = 8

# row-groups: list of (num_row_chunks, engine_name). Sum must be NT.
# NOTE: only DVE can sort — walrus rejects min/max TensorTensor on Pool.
GROUPS = [(NT, "vector")]

# debug knob: limit number of sort stages emitted (None = all)
STAGE_LIMIT = None
# benchmark knob: emit the whole kernel body this many times (timing only)
REPEAT = 1

FP = mybir.dt.float32
SD = mybir.dt.bfloat16  # sort dtype (bf16 -> DVE 2x perf mode on most stages)

# input names that are per-core (sharded); everything else is replicated
_SHARDED = {"xnat"}


def _sort_stages(n):
    """Uniform-direction bitonic network: (kind, param) list. 66 stages for
    n=2048. The "std" stages keep every operand AP at <=2 free dims with
    forward unit steps, which is required for the DVE bf16 2x perf mode
    (measured: reversed reads and 3-free-dim APs both drop to 1x)."""
    stages = []
    size = 2
    while size <= n:
        stages.append(("mirror", size))
        st = size // 4
        while st >= 1:
            stages.append(("std", st))
            st //= 2
        size *= 2
    return stages


def _emit_sort_stage(eng, kind, val, cur, oth):
    """One compare-exchange stage: read cur AP, write oth AP (same shape)."""
    if kind == "mirror":
        s = val
        half = s // 2
        v = cur.rearrange("p (n s) -> p n s", s=s)
        o = oth.rearrange("p (n s) -> p n s", s=s)
        lo, up = v[:, :, :half], v[:, :, half:]
        olo, oup = o[:, :, :half], o[:, :, half:]
        eng.tensor_tensor(olo, lo, up[:, :, ::-1], op=mybir.AluOpType.min)
        eng.tensor_tensor(oup, lo[:, :, ::-1], up, op=mybir.AluOpType.max)
    else:
        st = val
        v = cur.rearrange("p (n t s) -> p n t s", t=2, s=st)
        o = oth.rearrange("p (n t s) -> p n t s", t=2, s=st)
        eng.tensor_tensor(o[:, :, 0, :], v[:, :, 0, :], v[:, :, 1, :],
                          op=mybir.AluOpType.min)
        eng.tensor_tensor(o[:, :, 1, :], v[:, :, 0, :], v[:, :, 1, :],
                          op=mybir.AluOpType.max)


def _build_kernel():
    assert sum(g for g, _ in GROUPS) == NT
    nc = bacc.Bacc()

    xnat = nc.declare_dram_parameter("xnat", [N, D], SD, isOutput=False)   # X[b]
    thn = nc.declare_dram_parameter("thn", [P, D], SD, isOutput=False)     # theta_w
    wq = nc.declare_dram_parameter("wq", [P, M], SD, isOutput=False)       # weight
    tv = nc.declare_dram_parameter("tv", [128, M], FP, isOutput=False)     # t[m]
    eye = nc.declare_dram_parameter("eye", [128, 128], SD, isOutput=False)
    rw = nc.declare_dram_parameter("rw", [128, NT], FP, isOutput=False)    # rw[p]
    out = nc.declare_dram_parameter("out", [128, NT], FP, isOutput=True)

    stages = _sort_stages(N)
    if STAGE_LIMIT is not None:
        stages = stages[:STAGE_LIMIT]

    with TileContext(nc) as tc:
        with (
            tc.tile_pool(name="const", bufs=1) as const_pool,
            tc.tile_pool(name="xt", bufs=1) as xt_pool,
            tc.tile_pool(name="sa", bufs=1) as a_pool,
            tc.tile_pool(name="sb", bufs=1) as b_pool,
            tc.tile_pool(name="wq", bufs=2) as w_pool,
            tc.tile_pool(name="w2", bufs=1) as w2_pool,
            tc.tile_pool(name="prod", bufs=2) as prod_pool,
            tc.tile_pool(name="pst", bufs=2, space="PSUM") as pst_pool,
            tc.tile_pool(name="ps", bufs=1, space="PSUM") as psum_pool,
        ):
            eye_raw = const_pool.tile([128, 128], SD, tag="eyer")
            eye_sb = const_pool.tile([128, 128], SD, tag="eye")
            tv_sb = const_pool.tile([128, M], FP, tag="tv")
            tvb_sb = const_pool.tile([128, M], FP, tag="tvb")
            rw_sb = const_pool.tile([128, NT], FP, tag="rw")
            acc_sb = const_pool.tile([128, NT], FP, tag="acc")
            out_sb = const_pool.tile([128, NT], FP, tag="outsb")
            out_tmp = const_pool.tile([128, NT], FP, tag="outtmp")
            xn_sb = xt_pool.tile([128, N], SD, tag="xn")        # X[b] natural
            xt_sb = xt_pool.tile([D, N], SD, tag="xt")          # X[b].T
            thn_sb = xt_pool.tile([128, P], SD, tag="thn")      # theta_w natural
            tht_sb = xt_pool.tile([D, P], SD, tag="tht")        # theta_w.T

            nc.sync.dma_start(out=eye_raw[:], in_=eye[:])
            nc.sync.dma_start(out=tv_sb[:], in_=tv[:])
            nc.sync.dma_start(out=rw_sb[:], in_=rw[:])
            # X[b] as [128, 16*128]: xn_sb[p, k*128+d] = X[k*128+p, d]
            nc.sync.dma_start(
                out=xn_sb.rearrange("p (k d) -> p k d", d=D),
                in_=xnat.rearrange("(k p) d -> p k d", p=128))
            # theta_w as [128, 8*128]: thn_sb[p, r*128+d] = theta_w[r*128+p, d]
            nc.sync.dma_start(
                out=thn_sb.rearrange("p (r d) -> p r d", d=D),
                in_=thn.rearrange("(r p) d -> p r d", p=128))
            # Bounce eye through ACT so Matmult (transpose) instructions never
            # carry two DMA-queue semaphore waits (walrus codegen limit).
            nc.scalar.copy(out=eye_sb[:], in_=eye_raw[:])
            # bounce tv through ACT so gpsimd W2-build ops carry at most one
            # DMA-queue semaphore wait
            nc.scalar.copy(out=tvb_sb[:], in_=tv_sb[:])

            # on-device transposes: X[b].T [D, N] and theta_w.T [D, P]
            for k in range(N // 128):
                ps = pst_pool.tile([128, 128], SD, tag="pst", name="pst")
                nc.tensor.transpose(
                    ps[:], xn_sb[:, k * 128:(k + 1) * 128], eye_sb[:])
                nc.scalar.copy(out=xt_sb[:, k * 128:(k + 1) * 128], in_=ps[:])
            for r in range(NT):
                ps = pst_pool.tile([128, 128], SD, tag="pst", name="pst")
                nc.tensor.transpose(
                    ps[:], thn_sb[:, r * 128:(r + 1) * 128], eye_sb[:])
                nc.scalar.copy(out=tht_sb[:, r * 128:(r + 1) * 128], in_=ps[:])

            # W2 rows prebuilt on GPSIMD (overlaps the sort): even slots
            # weight*(1-t), odd slots weight*t
            w2_full = w2_pool.tile([128, NT * N], FP, tag="w2full")
            for t in range(NT):
                w_sb = w_pool.tile([128, M], SD, tag="wq", name="wq")
                nc.sync.dma_start(out=w_sb[:],
                                  in_=wq[t * 128:(t + 1) * 128, :])
                w2v = w2_full[:, t * N:(t + 1) * N].rearrange(
                    "p (m q) -> p m q", q=2)
                nc.gpsimd.tensor_mul(w2v[:, :, 1], w_sb[:], tvb_sb[:])
                nc.gpsimd.tensor_sub(w2v[:, :, 0], w_sb[:], w2v[:, :, 1])

            a_tiles, b_tiles = {}, {}
            for gi, (gsz, eng_name) in enumerate(GROUPS):
                sd = SD if eng_name == "vector" else FP
                a_tiles[gi] = a_pool.tile([128, gsz * N], sd,
                                          tag=f"a{gi}", name=f"a{gi}")
                b_tiles[gi] = b_pool.tile([128, gsz * N], sd,
                                          tag=f"b{gi}", name=f"b{gi}")

            def emit_body(rep_i):
                # ---- phase A: projection matmuls for every chunk (PE+ACT) --
                t0 = 0
                for gi, (gsz, _) in enumerate(GROUPS):
                    a_t = a_tiles[gi]
                    for r in range(gsz):
                        t = t0 + r
                        ps = psum_pool.tile([128, N], FP, tag="ps", name="ps")
                        for ch in range(N // MM_CHUNK):
                            nc.tensor.matmul(
                                ps[:, ch * MM_CHUNK:(ch + 1) * MM_CHUNK],
                                lhsT=tht_sb[:, t * 128:(t + 1) * 128],
                                rhs=xt_sb[:, ch * MM_CHUNK:(ch + 1) * MM_CHUNK],
                                start=True, stop=True,
                            )
                        nc.scalar.copy(out=a_t[:, r * N:(r + 1) * N], in_=ps[:])
                    t0 += gsz

                # ---- phase B: per-group sort + weighted reduction ----------
                # First and last stages are emitted per row-chunk so the sort
                # can start as soon as each chunk's matmul lands, and each
                # chunk's weighted reduction can start right after its own
                # final compare-exchange.
                t0 = 0
                for gi, (gsz, eng_name) in enumerate(GROUPS):
                    a_t, b_t = a_tiles[gi], b_tiles[gi]
                    eng = getattr(nc, eng_name)
                    cur, oth = a_t[:], b_t[:]
                    for si, stg in enumerate(stages):
                        first, last = si == 0, si == len(stages) - 1
                        if first or last:
                            for r in range(gsz):
                                sl = slice(r * N, (r + 1) * N)
                                _emit_sort_stage(eng, stg[0], stg[1],
                                                 cur[:, sl], oth[:, sl])
                                if not last:
                                    continue
                                t = t0 + r
                                w2sl = w2_full[:, t * N:(t + 1) * N]
                                # HW-proven finals: gpsimd mult + ACT accum
                                # (tensor_tensor_reduce compiles but crashes
                                # the core at runtime)
                                prod = prod_pool.tile([128, N], FP,
                                                      tag="prod",
                                                      name="prod")
                                nc.gpsimd.tensor_mul(prod[:], oth[:, sl],
                                                     w2sl)
                                nc.scalar.activation(
                                    cur[:, sl], prod[:],
                                    mybir.ActivationFunctionType.Copy,
                                    accum_out=acc_sb[:, t:t + 1])
                        else:
                            _emit_sort_stage(eng, stg[0], stg[1], cur, oth)
                        cur, oth = oth, cur
                    t0 += gsz

                # accumulate across repeat bodies so none is dead code;
                # the final output is REPEAT * (rw - acc), divided on host
                if rep_i == 0:
                    nc.vector.tensor_sub(out_sb[:], rw_sb[:], acc_sb[:])
                else:
                    nc.vector.tensor_sub(out_tmp[:], rw_sb[:], acc_sb[:])
                    nc.vector.tensor_add(out_sb[:], out_sb[:], out_tmp[:])

            for _rep in range(REPEAT):
                emit_body(_rep)
            nc.sync.dma_start(out=out[:], in_=out_sb[:])

    return nc


_NC_CACHE = None


def _get_nc():
    global _NC_CACHE
    if _NC_CACHE is None:
        nc = _build_kernel()
        nc.finalize()   # Bacc: runs wait-splitting + register allocation
        _NC_CACHE = nc
    return _NC_CACHE


def _host_precompute(X, theta_w, ref, weight):
    """Global (all-core) input arrays, keyed by dram parameter name."""
    X = np.ascontiguousarray(np.asarray(X, dtype=np.float32))
    theta_w = np.ascontiguousarray(np.asarray(theta_w, dtype=np.float32))
    ref = np.asarray(ref, dtype=np.float32)
    weight = np.ascontiguousarray(np.asarray(weight, dtype=np.float32))

    x1d = np.linspace(0.0, 1.0, N + 2, dtype=np.float32)[1:-1]
    xnew = np.linspace(0.0, 1.0, M + 2, dtype=np.float32)[1:-1]
    ind = 2 * np.arange(M)      # == clip(searchsorted(x1d, xnew) - 1, 0, N-2)
    eps = np.float32(np.finfo(np.float32).eps)
    dx = x1d[1:] - x1d[:-1]
    t = ((xnew - x1d[ind]) / (eps + dx[ind])).astype(np.float32)

    rw = (weight.astype(np.float64) @ ref.astype(np.float64)).astype(np.float32)

    import ml_dtypes
    bf = ml_dtypes.bfloat16
    return {
        "xnat": X.reshape(B * N, D).astype(bf),           # sharded over cores
        "thn": theta_w.astype(bf),
        "wq": weight.astype(bf),
        "tv": np.ascontiguousarray(np.broadcast_to(t, (128, M))),
        "eye": np.eye(128, dtype=np.float32).astype(bf),
        "rw": np.ascontiguousarray(rw.reshape(NT, 128).T),
    }


# ---------------------------------------------------------------------------
# Cached-jit executor: trace/compile once per process, then each call is
# host->device of ~12 MB + one dispatch. Replicated inputs use P(None) so a
# single copy is shipped rather than 8.
# ---------------------------------------------------------------------------

_EXEC_CACHE = {}


def _get_exec():
    key = REPEAT
    if key in _EXEC_CACHE:
        return _EXEC_CACHE[key]
    import jax
    from jax.sharding import Mesh, PartitionSpec
    from jax.experimental.shard_map import shard_map
    from concourse import mybir as _mybir
    from concourse.bass2jax import (
        _bass_exec_p, install_neuronx_cc_hook, partition_id_tensor,
    )

    install_neuronx_cc_hook()
    nc = _get_nc()

    partition_name = (nc.partition_id_tensor.name
                      if nc.partition_id_tensor else None)
    in_names, out_names, out_avals, zero_outs = [], [], [], []
    for alloc in nc.m.functions[0].allocations:
        if not isinstance(alloc, _mybir.MemoryLocationSet):
            continue
        name = alloc.memorylocations[0].name
        if alloc.kind == "ExternalInput":
            if name == partition_name:
                continue
            in_names.append(name)
        elif alloc.kind == "ExternalOutput":
            out_names.append(name)
            shape = tuple(alloc.tensor_shape)
            dtype = _mybir.dt.np(alloc.dtype)
            out_avals.append(jax.core.ShapedArray(shape, dtype))
            zero_outs.append(np.zeros(shape, dtype))
    n_params = len(in_names)
    all_names = in_names + out_names
    if partition_name is not None:
        all_names = all_names + [partition_name]

    def _body(*args):
        operands = list(args)
        if partition_name is not None:
            operands.append(partition_id_tensor())
        outs = _bass_exec_p.bind(
            *operands,
            out_avals=tuple(out_avals),
            in_names=tuple(all_names),
            out_names=tuple(out_names),
            lowering_input_output_aliases=(),
            sim_require_finite=True,
            sim_require_nnan=True,
            nc=nc,
        )
        return tuple(outs)

    devices = jax.devices()[:N_CORES]
    mesh = Mesh(np.asarray(devices), ("core",))
    in_specs = tuple(
        PartitionSpec("core") if nm in _SHARDED else PartitionSpec()
        for nm in in_names
    ) + (PartitionSpec("core"),) * len(zero_outs)
    out_specs = (PartitionSpec("core"),) * len(out_names)
    fn = jax.jit(
        shard_map(_body, mesh=mesh,
                  in_specs=in_specs, out_specs=out_specs,
                  check_rep=False),
        keep_unused=True,
    )
    shardings = [jax.sharding.NamedSharding(mesh, sp) for sp in in_specs]
    res = (fn, in_names, out_names, zero_outs, shardings)
    _EXEC_CACHE[key] = res
    return res


def _run_jit(glob_inputs):
    import jax
    fn, in_names, out_names, zero_outs, shardings = _get_exec()
    args = [glob_inputs[nm] for nm in in_names]
    args += [np.zeros((N_CORES * z.shape[0], *z.shape[1:]), z.dtype)
             for z in zero_outs]
    # one batched H2D (per-array puts each pay an axon roundtrip)
    try:
        dargs = jax.device_put(args, shardings)
    except Exception:
        dargs = args
    outs = fn(*dargs)
    jax.block_until_ready(outs)
    return {nm: np.asarray(o) for nm, o in zip(out_names, outs)}


def _run_fallback(glob_inputs):
    """Per-core run via run_bass_kernel_spmd (native-NRT capable path)."""
    from concourse.bass_utils import run_bass_kernel_spmd
    nc = _get_nc()
    in_maps = []
    for c in range(N_CORES):
        m = {}
        for nm, arr in glob_inputs.items():
            if nm in _SHARDED:
                sh = arr.shape[0] // N_CORES
                m[nm] = arr[c * sh:(c + 1) * sh]
            else:
                m[nm] = arr
        in_maps.append(m)
    res = run_bass_kernel_spmd(nc, in_maps, list(range(N_CORES)))
    outs = res.results if hasattr(res, "results") else res
    return {"out": np.concatenate([o["out"] for o in outs], axis=0)}


def _assemble(out_concat):
    out_full = np.empty((B, P), dtype=np.float32)
    for c in range(N_CORES):
        o = out_concat[c * 128:(c + 1) * 128]
        out_full[c] = np.ascontiguousarray(o.T).reshape(P)
    return out_full / REPEAT


def kernel(X, theta_w, ref, weight):
    import time as _time

    glob_inputs = _host_precompute(X, theta_w, ref, weight)
    last_err = None
    for attempt in range(3):
        try:
            outs = _run_jit(glob_inputs)
            return _assemble(outs["out"])
        except Exception as e:  # transient transport errors (mesh desync)
            last_err = e
            _time.sleep(3)
    # final fallback: sanctioned spmd runner (works native or axon)
    try:
        outs = _run_fallback(glob_inputs)
        return _assemble(outs["out"])
    except Exception:
        raise last_err


# ---------------------------------------------------------------------------
# Benchmark path: cached jit + device-resident inputs, excludes host transfer.
# ---------------------------------------------------------------------------

def make_bench(X, theta_w, ref, weight):
    import jax

    fn, in_names, out_names, zero_outs, _shardings = _get_exec()
    glob_inputs = _host_precompute(X, theta_w, ref, weight)
    args = [glob_inputs[nm] for nm in in_names]
    args += [np.zeros((N_CORES * z.shape[0], *z.shape[1:]), z.dtype)
             for z in zero_outs]
    dev_in = [jax.device_put(a) for a in args]

    def run():
        outs = fn(*dev_in)
        jax.block_until_ready(outs)
        return outs

    def collect(outs):
        return _assemble(np.asarray(outs[0]))

    return run, collect
